# revision 1
# baseline (speedup 1.0000x reference)
"""Trainium2 Bass kernel for CommunityPassing (segment mean + gather).

Algorithm (8 NeuronCores, data-parallel over nodes):
  host: shard x/community over 8 cores along the node axis; within each
        shard, stably sort node indices by community id and pack them into
        128-row tiles grouped by community "chunk" (128 communities per
        chunk, 8 chunks for 1000 communities). Pad each (core, chunk)
        block to a shared tile count so all cores run one SPMD program.
  dev:  phase 1 - stream sorted x tiles; build a per-tile one-hot
        selection matrix B[node, local_comm] with a DVE is_equal against
        an iota row; matmul B^T @ x_tile accumulating into a PSUM tile
        per community chunk -> per-core partial community sums.
        AllReduce the [1024, 256] partial sums across the 8 cores,
        multiply by host-computed 1/count, write the [1024, 256]
        community-mean table to DRAM.
        phase 2 - dma_gather rows of the table with the original-order
        community ids (int16) and stream the result to the output.
  host: concatenate the 8 output shards.
"""

import os
import sys

import numpy as np

for _p in ("/opt/trn_rl_repo", "/opt/pypackages"):
    if _p not in sys.path and os.path.isdir(_p):
        sys.path.append(_p)

# Problem constants (hardcoded per the task contract).
N_FULL = 500000
F = 256
NUM_COMMS = 1000
EPS = 1e-12
M = 8               # cores
P = 128             # partitions
NC_CHUNKS = 8       # community chunks of 128 (8*128 = 1024 >= 1000)
GATHER_BATCH = 2048  # rows per dma_gather (multiple of 128)
XB = 8              # x tiles per streaming DMA (8 * 128KB = 1MB)
JB = GATHER_BATCH // P

# Stash of the most recent run's BassKernelResults (for test harnesses).
LAST_RESULTS = None


def _host_prep(x, community):
    """Build per-core device inputs. Returns (in_maps, plan)."""
    x = np.ascontiguousarray(np.asarray(x, dtype=np.float32))
    community = np.asarray(community).astype(np.int64)
    n = x.shape[0]
    assert n % M == 0
    nl = n // M

    comm_sh = community.reshape(M, nl)
    perms = np.argsort(comm_sh, axis=1, kind="stable")
    comm_sorted = np.take_along_axis(comm_sh, perms, axis=1)

    # per (core, chunk) node counts
    chunk_ids = comm_sorted >> 7  # // 128
    cnts = np.zeros((M, NC_CHUNKS), dtype=np.int64)
    for m in range(M):
        bc = np.bincount(chunk_ids[m], minlength=NC_CHUNKS)
        cnts[m] = bc[:NC_CHUNKS]
    t_k = np.maximum(1, -(-cnts.max(axis=0) // P))  # ceil, shared by all cores
    t_total = int(t_k.sum())
    chunk_of_tile = np.repeat(np.arange(NC_CHUNKS), t_k)
    tile_off = np.concatenate([[0], np.cumsum(t_k)])  # tile index base per chunk

    # counts -> 1/max(cnt, eps), [p, k] layout (community id = k*128 + p)
    cnt_full = np.bincount(community, minlength=NUM_COMMS).astype(np.float32)
    inv_pad = np.zeros((NC_CHUNKS * P,), np.float32)
    inv_pad[:NUM_COMMS] = 1.0 / np.maximum(cnt_full, np.float32(EPS))
    invc = np.ascontiguousarray(inv_pad.reshape(NC_CHUNKS, P).T)  # [128, 8]

    iota = np.ascontiguousarray(
        np.tile(np.arange(P, dtype=np.float32), (P, 1))
    )  # [128, 128], each row 0..127
    import ml_dtypes

    ident = np.eye(P).astype(ml_dtypes.bfloat16)

    in_maps = []
    origs = []
    for m in range(M):
        x_m = x[m * nl : (m + 1) * nl]
        xs = np.zeros((t_total * P, F), dtype=np.float32)
        locid = np.full((t_total * P,), -1.0, dtype=np.float32)
        orig = np.full((t_total * P,), -1, dtype=np.int64)
        start = 0
        for k in range(NC_CHUNKS):
            c = int(cnts[m, k])
            row = int(tile_off[k]) * P
            sel = perms[m, start : start + c]
            xs[row : row + c] = x_m[sel]
            orig[row : row + c] = sel
            locid[row : row + c] = comm_sorted[m, start : start + c] - k * P
            start += c
        locid_t = np.ascontiguousarray(locid.reshape(t_total, P).T)  # [128, T]
        origs.append(orig)

        xs_hi = xs.astype(ml_dtypes.bfloat16)
        xs_lo = (xs - xs_hi.astype(np.float32)).astype(ml_dtypes.bfloat16)
        in_maps.append(
            {
                "xs_hi": xs_hi,
                "xs_lo": xs_lo,
                "locid": locid_t,
                "iota": iota,
                "ident": ident,
                "invc": invc,
            }
        )

    plan = {
        "nl": nl,
        "t_k": [int(v) for v in t_k],
        "t_total": t_total,
        "chunk_of_tile": [int(v) for v in chunk_of_tile],
        "tile_off": [int(v) for v in tile_off],
        "origs": origs,
    }
    return in_maps, plan


def _build_program(plan, use_collective=True, use_gather=True):
    from concourse import bacc, mybir, tile

    t_total = plan["t_total"]
    chunk_of_tile = plan["chunk_of_tile"]
    tile_off = plan["tile_off"]

    dt = mybir.dt
    nc = bacc.Bacc("TRN2", target_bir_lowering=False, debug=False, num_devices=M)

    xs_hi = nc.dram_tensor("xs_hi", [t_total * P, F], dt.bfloat16, kind="ExternalInput")
    xs_lo = nc.dram_tensor("xs_lo", [t_total * P, F], dt.bfloat16, kind="ExternalInput")
    locid = nc.dram_tensor("locid", [P, t_total], dt.float32, kind="ExternalInput")
    iota = nc.dram_tensor("iota", [P, P], dt.float32, kind="ExternalInput")
    ident = nc.dram_tensor("ident", [P, P], dt.bfloat16, kind="ExternalInput")
    invc = nc.dram_tensor("invc", [P, NC_CHUNKS], dt.float32, kind="ExternalInput")
    out = nc.dram_tensor("out", [t_total * P, F], dt.float32, kind="ExternalOutput")

    xs_hi_view = xs_hi.ap().rearrange("(t p) f -> p t f", p=P)  # [128, T, 256]
    xs_lo_view = xs_lo.ap().rearrange("(t p) f -> p t f", p=P)

    with tile.TileContext(nc) as tc:
        with (
            tc.tile_pool(name="const", bufs=1) as constp,
            tc.tile_pool(name="xsp", bufs=3) as xsp,
            tc.tile_pool(name="bp", bufs=6) as bp,
            tc.tile_pool(name="acc", bufs=1) as accp,
            tc.tile_pool(name="psum", bufs=2, space="PSUM") as psp,
            tc.tile_pool(name="dram", bufs=1, space="DRAM") as dramp,
        ):
            iota_t = constp.tile([P, P], dt.float32)
            nc.sync.dma_start(out=iota_t[:], in_=iota.ap())
            ident_t = constp.tile([P, P], dt.bfloat16)
            nc.sync.dma_start(out=ident_t[:], in_=ident.ap())
            locid_t = constp.tile([P, t_total], dt.float32)
            nc.sync.dma_start(out=locid_t[:], in_=locid.ap())
            invc_t = constp.tile([P, NC_CHUNKS], dt.float32)
            nc.sync.dma_start(out=invc_t[:], in_=invc.ap())

            comm_sum = accp.tile([P, NC_CHUNKS * F], dt.float32)

            # ---- phase 1: streamed one-hot matmul segment sums ----
            xsb_hi = None
            xsb_lo = None
            bstart = 0
            psum_t = None
            for t in range(t_total):
                if t % XB == 0:
                    bstart = t
                    w = min(XB, t_total - t)
                    xsb_hi = xsp.tile([P, XB * F], dt.bfloat16, tag="xsbh")
                    nc.sync.dma_start(
                        out=xsb_hi[:, : w * F].rearrange("p (b f) -> p b f", b=w),
                        in_=xs_hi_view[:, t : t + w, :],
                    )
                    xsb_lo = xsp.tile([P, XB * F], dt.bfloat16, tag="xsbl")
                    nc.sync.dma_start(
                        out=xsb_lo[:, : w * F].rearrange("p (b f) -> p b f", b=w),
                        in_=xs_lo_view[:, t : t + w, :],
                    )
                k = chunk_of_tile[t]
                first = t == tile_off[k]
                last = t == tile_off[k + 1] - 1
                if first:
                    psum_t = psp.tile([P, F], dt.float32)
                b_t = bp.tile([P, P], dt.bfloat16, tag="b")
                nc.vector.tensor_scalar(
                    b_t[:],
                    iota_t[:],
                    locid_t[:, t : t + 1],
                    None,
                    mybir.AluOpType.is_equal,
                )
                j = t - bstart
                nc.tensor.matmul(
                    psum_t[:],
                    lhsT=b_t[:],
                    rhs=xsb_hi[:, j * F : (j + 1) * F],
                    start=first,
                    stop=False,
                )
                nc.tensor.matmul(
                    psum_t[:],
                    lhsT=b_t[:],
                    rhs=xsb_lo[:, j * F : (j + 1) * F],
                    start=False,
                    stop=last,
                )
                if last:
                    nc.vector.tensor_copy(
                        out=comm_sum[:, k * F : (k + 1) * F], in_=psum_t[:]
                    )

            # ---- all-reduce partial sums, scale by 1/count, write table ----
            ar_in = dramp.tile([P, NC_CHUNKS * F], dt.float32)
            ar_out = dramp.tile([P, NC_CHUNKS * F], dt.float32)
            nc.sync.dma_start(out=ar_in, in_=comm_sum[:])
            if use_collective:
                nc.gpsimd.collective_compute(
                    "AllReduce",
                    mybir.AluOpType.add,
                    replica_groups=[list(range(M))],
                    ins=[ar_in.opt()],
                    outs=[ar_out.opt()],
                )
            else:
                nc.sync.dma_start(out=ar_out, in_=ar_in)
            mean_sb = accp.tile([P, NC_CHUNKS * F], dt.float32)
            nc.sync.dma_start(out=mean_sb[:], in_=ar_out)
            for k in range(NC_CHUNKS):
                nc.vector.tensor_scalar(
                    mean_sb[:, k * F : (k + 1) * F],
                    mean_sb[:, k * F : (k + 1) * F],
                    invc_t[:, k : k + 1],
                    None,
                    mybir.AluOpType.mult,
                )

            # ---- phase 2: broadcast means back to (sorted) nodes ----
            # out_tile[node, f] = B[node, comm] @ mean_chunk[comm, f];
            # matmul wants lhsT = B^T, produced by a PE transpose.
            # fp32 matmul streams at 1/4 rate, so split the mean into two
            # bf16 limbs (hi + residual) and run two full-rate bf16 matmuls
            # accumulating in fp32 PSUM (~16-bit-exact result).
            mean_hi = accp.tile([P, NC_CHUNKS * F], dt.bfloat16)
            mean_lo = accp.tile([P, NC_CHUNKS * F], dt.bfloat16)
            mean_rest = accp.tile([P, NC_CHUNKS * F], dt.float32)
            nc.vector.tensor_copy(out=mean_hi[:], in_=mean_sb[:])
            nc.vector.tensor_copy(out=mean_rest[:], in_=mean_hi[:])
            nc.vector.tensor_tensor(
                out=mean_rest[:],
                in0=mean_sb[:],
                in1=mean_rest[:],
                op=mybir.AluOpType.subtract,
            )
            nc.vector.tensor_copy(out=mean_lo[:], in_=mean_rest[:])
            out_view = out.ap().rearrange("(t p) f -> p t f", p=P)
            with (
                tc.tile_pool(name="pst", bufs=2, space="PSUM") as pst,
                tc.tile_pool(name="pso", bufs=2, space="PSUM") as pso,
                tc.tile_pool(name="btp", bufs=4) as btp,
                tc.tile_pool(name="outp", bufs=3) as outp,
            ):
                if not use_gather:
                    t_total2 = 0
                else:
                    t_total2 = t_total
                outsb = None
                for t in range(t_total2):
                    if t % XB == 0:
                        outsb = outp.tile([P, XB * F], dt.float32, tag="outsb")
                    k = chunk_of_tile[t]
                    b2 = bp.tile([P, P], dt.bfloat16, tag="b2")
                    nc.vector.tensor_scalar(
                        b2[:],
                        iota_t[:],
                        locid_t[:, t : t + 1],
                        None,
                        mybir.AluOpType.is_equal,
                    )
                    bt_ps = pst.tile([P, P], dt.bfloat16)
                    nc.tensor.transpose(out=bt_ps[:], in_=b2[:], identity=ident_t[:])
                    bt_sb = btp.tile([P, P], dt.bfloat16, tag="bt")
                    nc.scalar.copy(out=bt_sb[:], in_=bt_ps[:])
                    op_ps = pso.tile([P, F], dt.float32)
                    nc.tensor.matmul(
                        op_ps[:],
                        lhsT=bt_sb[:],
                        rhs=mean_hi[:, k * F : (k + 1) * F],
                        start=True,
                        stop=False,
                    )
                    nc.tensor.matmul(
                        op_ps[:],
                        lhsT=bt_sb[:],
                        rhs=mean_lo[:, k * F : (k + 1) * F],
                        start=False,
                        stop=True,
                    )
                    j = t % XB
                    nc.vector.tensor_copy(
                        out=outsb[:, j * F : (j + 1) * F], in_=op_ps[:]
                    )
                    if t % XB == XB - 1 or t == t_total2 - 1:
                        t0 = t - j
                        w = j + 1
                        nc.sync.dma_start(
                            out=out_view[:, t0 : t0 + w, :],
                            in_=outsb[:, : w * F].rearrange(
                                "p (b f) -> p b f", b=w
                            ),
                        )

    nc.compile()
    return nc


def kernel(x, community):
    global LAST_RESULTS
    from concourse.bass_utils import run_bass_kernel_spmd

    in_maps, plan = _host_prep(x, community)
    nc = _build_program(plan)
    res = run_bass_kernel_spmd(nc, in_maps, core_ids=list(range(M)))
    LAST_RESULTS = res
    nl = plan["nl"]
    outs = []
    for m in range(M):
        out_sorted = res.results[m]["out"]
        orig = plan["origs"][m]
        valid = orig >= 0
        out_m = np.empty((nl, F), dtype=np.float32)
        out_m[orig[valid]] = out_sorted[valid]
        outs.append(out_m)
    return np.concatenate(outs, axis=0)



# revision 4
# speedup vs baseline: 2.3361x; 2.3361x over previous
"""Trainium2 Bass kernel for CommunityPassing (segment mean + gather).

Algorithm (8 NeuronCores, data-parallel over nodes):
  host: shard x/community over 8 cores along the node axis; within each
        shard, stably sort node indices by community id and pack them into
        128-row tiles grouped by community "chunk" (128 communities per
        chunk, 8 chunks for 1000 communities). Pad each (core, chunk)
        block to a shared tile count so all cores run one SPMD program.
        Precompute per-tile one-hot B^T (comm-major) in fp8 (exact for
        0/1) plus local comm ids and 1/count.
  dev:  phase 1 - stream sorted x tiles (bf16); build per-tile one-hot
        B[node, comm] with a DVE is_equal; matmul B^T(!)=lhsT @ x_tile
        accumulating into a PSUM tile per community chunk.
        Every 2 chunks, AllReduce the [128, 512] partial sums across the
        8 cores (overlapped with remaining phase-1 work), scale by
        host-computed 1/count, convert to bf16 mean table.
        phase 2 - per tile, matmul lhsT=B^T (fp8, from host) with
        rhs=mean_chunk [128, 256] -> out rows in PSUM; copy to bf16
        staging (Scalar/GpSimd engines) and stream to DRAM.
  host: upcast bf16 -> fp32, unsort, concatenate the 8 output shards.
"""

import os
import sys

import numpy as np

for _p in ("/opt/trn_rl_repo", "/opt/pypackages"):
    if _p not in sys.path and os.path.isdir(_p):
        sys.path.append(_p)

# Problem constants (hardcoded per the task contract).
N_FULL = 500000
F = 256
NUM_COMMS = 1000
EPS = 1e-12
M = 8               # cores
P = 128             # partitions
NC_CHUNKS = 8       # community chunks of 128 (8*128 = 1024 >= 1000)
AR_CHUNKS = 2       # community chunks per all-reduce group
XB = 16             # tiles per streaming DMA batch (16*512B = 8KB lines)

# Stash of the most recent run's BassKernelResults (for test harnesses).
LAST_RESULTS = None


def _host_prep(x, community):
    """Build per-core device inputs. Returns (in_maps, plan)."""
    import ml_dtypes

    x = np.ascontiguousarray(np.asarray(x, dtype=np.float32))
    community = np.asarray(community).astype(np.int64)
    n = x.shape[0]
    assert n % M == 0
    nl = n // M

    comm_sh = community.reshape(M, nl)
    perms = np.argsort(comm_sh, axis=1, kind="stable")
    comm_sorted = np.take_along_axis(comm_sh, perms, axis=1)

    # per (core, chunk) node counts
    chunk_ids = comm_sorted >> 7  # // 128
    cnts = np.zeros((M, NC_CHUNKS), dtype=np.int64)
    for m in range(M):
        bc = np.bincount(chunk_ids[m], minlength=NC_CHUNKS)
        cnts[m] = bc[:NC_CHUNKS]
    t_k = np.maximum(1, -(-cnts.max(axis=0) // P))  # ceil, shared by all cores
    t_total = int(t_k.sum())
    chunk_of_tile = np.repeat(np.arange(NC_CHUNKS), t_k)
    tile_off = np.concatenate([[0], np.cumsum(t_k)])  # tile index base per chunk

    # counts -> 1/max(cnt, eps), [p, k] layout (community id = k*128 + p)
    cnt_full = np.bincount(community, minlength=NUM_COMMS).astype(np.float32)
    inv_pad = np.zeros((NC_CHUNKS * P,), np.float32)
    inv_pad[:NUM_COMMS] = 1.0 / np.maximum(cnt_full, np.float32(EPS))
    invc = np.ascontiguousarray(inv_pad.reshape(NC_CHUNKS, P).T)  # [128, 8]

    iota = np.ascontiguousarray(
        np.tile(np.arange(P, dtype=ml_dtypes.bfloat16), (P, 1))
    )  # [128, 128] bf16, each row 0..127

    arange_p = np.arange(P, dtype=np.int32)
    in_maps = []
    origs = []
    for m in range(M):
        x_m = x[m * nl : (m + 1) * nl]
        xs = np.zeros((t_total * P, F), dtype=np.float32)
        locid = np.full((t_total * P,), -1.0, dtype=np.float32)
        orig = np.full((t_total * P,), -1, dtype=np.int64)
        start = 0
        for k in range(NC_CHUNKS):
            c = int(cnts[m, k])
            row = int(tile_off[k]) * P
            sel = perms[m, start : start + c]
            xs[row : row + c] = x_m[sel]
            orig[row : row + c] = sel
            locid[row : row + c] = comm_sorted[m, start : start + c] - k * P
            start += c
        origs.append(orig)

        # xs: [128, T, 256] bf16 -- xs_dev[p, t, f] = x_sorted[t*128+p, f]
        xs_dev = np.ascontiguousarray(
            xs.reshape(t_total, P, F).transpose(1, 0, 2).astype(ml_dtypes.bfloat16)
        )
        # locid: [128, T] fp32 (per-partition scalar for phase-1 is_equal)
        locid_t = np.ascontiguousarray(locid.reshape(t_total, P).T)
        # BT: [128, T, 128] fp8e4 -- BT[c, t, j] = 1 iff locid[t*128+j] == c
        ls = locid.reshape(t_total, P).astype(np.int32)  # [T, 128]
        bt = (ls[None, :, :] == arange_p[:, None, None]).astype(
            ml_dtypes.float8_e4m3
        )
        in_maps.append(
            {
                "xs": xs_dev,
                "locid": locid_t,
                "bt": np.ascontiguousarray(bt),
                "iota": iota,
                "invc": invc,
            }
        )

    plan = {
        "nl": nl,
        "t_k": [int(v) for v in t_k],
        "t_total": t_total,
        "chunk_of_tile": [int(v) for v in chunk_of_tile],
        "tile_off": [int(v) for v in tile_off],
        "origs": origs,
    }
    return in_maps, plan


def _build_program(plan, use_collective=True):
    from concourse import bacc, mybir, tile

    t_total = plan["t_total"]
    chunk_of_tile = plan["chunk_of_tile"]
    tile_off = plan["tile_off"]

    dt = mybir.dt
    nc = bacc.Bacc("TRN2", target_bir_lowering=False, debug=False, num_devices=M)

    xs = nc.dram_tensor("xs", [P, t_total, F], dt.bfloat16, kind="ExternalInput")
    locid = nc.dram_tensor("locid", [P, t_total], dt.float32, kind="ExternalInput")
    btd = nc.dram_tensor("bt", [P, t_total, P], dt.float8e4, kind="ExternalInput")
    iota = nc.dram_tensor("iota", [P, P], dt.bfloat16, kind="ExternalInput")
    invc = nc.dram_tensor("invc", [P, NC_CHUNKS], dt.float32, kind="ExternalInput")
    out = nc.dram_tensor("out", [P, t_total, F], dt.bfloat16, kind="ExternalOutput")

    n_ar = NC_CHUNKS // AR_CHUNKS
    arw = AR_CHUNKS * F  # free width of one all-reduce group

    with tile.TileContext(nc) as tc:
        with (
            tc.tile_pool(name="const", bufs=1) as constp,
            tc.tile_pool(name="xsp", bufs=3) as xsp,
            tc.tile_pool(name="btp", bufs=3) as btp,
            tc.tile_pool(name="bp", bufs=8) as bp,
            tc.tile_pool(name="acc", bufs=1) as accp,
            tc.tile_pool(name="outp", bufs=3) as outp,
            tc.tile_pool(name="ps1", bufs=2, space="PSUM") as ps1,
            tc.tile_pool(name="ps2", bufs=4, space="PSUM") as ps2,
            tc.tile_pool(name="dram", bufs=1, space="DRAM") as dramp,
        ):
            iota_t = constp.tile([P, P], dt.bfloat16)
            nc.sync.dma_start(out=iota_t[:], in_=iota.ap())
            locid_t = constp.tile([P, t_total], dt.float32)
            nc.sync.dma_start(out=locid_t[:], in_=locid.ap())
            invc_t = constp.tile([P, NC_CHUNKS], dt.float32)
            nc.sync.dma_start(out=invc_t[:], in_=invc.ap())

            comm_sum = accp.tile([P, NC_CHUNKS * F], dt.float32)
            mean_f32 = accp.tile([P, NC_CHUNKS * F], dt.float32)
            mean_bf = accp.tile([P, NC_CHUNKS * F], dt.bfloat16)
            ar_in = [
                dramp.tile([P, arw], dt.float32, name=f"ar_in{g}")
                for g in range(n_ar)
            ]
            ar_out = [
                dramp.tile([P, arw], dt.float32, name=f"ar_out{g}")
                for g in range(n_ar)
            ]

            # ---- phase 1: streamed one-hot matmul segment sums ----
            xsb = None
            psum_t = None
            for t in range(t_total):
                if t % XB == 0:
                    w = min(XB, t_total - t)
                    xsb = xsp.tile([P, XB * F], dt.bfloat16, tag="xsb")
                    nc.sync.dma_start(
                        out=xsb[:, : w * F].rearrange("p (b f) -> p b f", b=w),
                        in_=xs.ap()[:, t : t + w, :],
                    )
                k = chunk_of_tile[t]
                first = t == tile_off[k]
                last = t == tile_off[k + 1] - 1
                if first:
                    psum_t = ps1.tile([P, F], dt.float32)
                b_t = bp.tile([P, P], dt.bfloat16, tag="b")
                nc.vector.tensor_scalar(
                    b_t[:],
                    iota_t[:],
                    locid_t[:, t : t + 1],
                    None,
                    mybir.AluOpType.is_equal,
                )
                j = t % XB
                nc.tensor.matmul(
                    psum_t[:],
                    lhsT=b_t[:],
                    rhs=xsb[:, j * F : (j + 1) * F],
                    start=first,
                    stop=last,
                )
                if last:
                    nc.scalar.copy(
                        out=comm_sum[:, k * F : (k + 1) * F], in_=psum_t[:]
                    )
                    if (k + 1) % AR_CHUNKS == 0:
                        g = k // AR_CHUNKS
                        lo = g * arw
                        nc.sync.dma_start(
                            out=ar_in[g], in_=comm_sum[:, lo : lo + arw]
                        )
                        if use_collective:
                            nc.gpsimd.collective_compute(
                                "AllReduce",
                                mybir.AluOpType.add,
                                replica_groups=[list(range(M))],
                                ins=[ar_in[g].opt()],
                                outs=[ar_out[g].opt()],
                            )
                            nc.sync.dma_start(
                                out=mean_f32[:, lo : lo + arw], in_=ar_out[g]
                            )
                        else:
                            nc.sync.dma_start(
                                out=mean_f32[:, lo : lo + arw], in_=ar_in[g]
                            )
                        for kk in range(g * AR_CHUNKS, (g + 1) * AR_CHUNKS):
                            nc.vector.tensor_scalar(
                                mean_bf[:, kk * F : (kk + 1) * F],
                                mean_f32[:, kk * F : (kk + 1) * F],
                                invc_t[:, kk : kk + 1],
                                None,
                                mybir.AluOpType.mult,
                            )

            # ---- phase 2: gather means back to (sorted) nodes ----
            # out_tile[node, f] = B^T.T @ mean_chunk; lhsT = B^T comes
            # precomputed from host in fp8 (exact for one-hot 0/1).
            btb = None
            outsb = None
            pso = None
            for t in range(t_total):
                if t % XB == 0:
                    w = min(XB, t_total - t)
                    btb = btp.tile([P, XB * P], dt.float8e4, tag="btb")
                    nc.sync.dma_start(
                        out=btb[:, : w * P].rearrange("p (b c) -> p b c", b=w),
                        in_=btd.ap()[:, t : t + w, :],
                    )
                    outsb = outp.tile([P, XB * F], dt.bfloat16, tag="outsb")
                k = chunk_of_tile[t]
                j = t % XB
                half = t % 2
                if half == 0:
                    pso = ps2.tile([P, 2 * F], dt.float32)
                nc.tensor.matmul(
                    pso[:, half * F : (half + 1) * F],
                    lhsT=btb[:, j * P : (j + 1) * P],
                    rhs=mean_bf[:, k * F : (k + 1) * F],
                    start=True,
                    stop=True,
                )
                pair_end = half == 1 or t == t_total - 1
                if pair_end:
                    wcols = (half + 1) * F
                    dst = outsb[:, (j - half) * F : (j - half) * F + wcols]
                    if (t // 2) % 2 == 0:
                        nc.scalar.copy(out=dst, in_=pso[:, :wcols])
                    else:
                        nc.vector.tensor_copy(out=dst, in_=pso[:, :wcols])
                if j == XB - 1 or t == t_total - 1:
                    t0 = t - j
                    w = j + 1
                    nc.sync.dma_start(
                        out=out.ap()[:, t0 : t0 + w, :],
                        in_=outsb[:, : w * F].rearrange("p (b f) -> p b f", b=w),
                    )

    nc.compile()
    return nc


def kernel(x, community):
    global LAST_RESULTS
    from concourse.bass_utils import run_bass_kernel_spmd

    in_maps, plan = _host_prep(x, community)
    nc = _build_program(plan)
    res = run_bass_kernel_spmd(nc, in_maps, core_ids=list(range(M)))
    LAST_RESULTS = res
    nl = plan["nl"]
    t_total = plan["t_total"]
    outs = []
    for m in range(M):
        od = res.results[m]["out"]  # [128, T, 256] bf16
        out_sorted = (
            np.asarray(od).transpose(1, 0, 2).reshape(t_total * P, F)
        ).astype(np.float32)
        orig = plan["origs"][m]
        valid = orig >= 0
        out_m = np.empty((nl, F), dtype=np.float32)
        out_m[orig[valid]] = out_sorted[valid]
        outs.append(out_m)
    return np.concatenate(outs, axis=0)


# revision 5
# speedup vs baseline: 2.6283x; 1.1251x over previous
"""Trainium2 Bass kernel for CommunityPassing (segment mean + gather).

Algorithm (8 NeuronCores, data-parallel over nodes):
  host: shard x/community over 8 cores along the node axis; within each
        shard, stably sort node indices by community id and pack them into
        128-row tiles grouped by community "chunk" (128 communities per
        chunk, 8 chunks for 1000 communities). Pad each (core, chunk)
        block to a shared tile count so all cores run one SPMD program.
        Precompute the one-hot B^T [comm, sorted_node] in fp8 (exact for
        0/1) plus per-tile local comm ids and 1/count.
  dev:  phase 1 - stream sorted x tiles (bf16); build per-tile one-hot
        B[node, comm] with a DVE is_equal; matmul lhsT=B @ x_tile
        accumulating into a PSUM tile per community chunk.
        Every 2 chunks, AllReduce the [128, 512] partial sums across the
        8 cores; the AR issue, the AR-result DMA and the 1/count scale
        all live on the GpSimd queue so no streaming engine blocks on
        the collective. Phase 1 keeps flowing underneath.
        phase 2 - per (chunk, feature-half), matmul lhsT=mean[c, f128]
        (stationary) with wide rhs=B^T[c, n512] fp8 slices -> out^T
        [f128, n] in PSUM; copy to bf16 staging (Scalar/Vector
        alternating) and stream out^T to DRAM.
  host: upcast bf16 -> fp32, transpose, unsort, concatenate the 8
        output shards.
"""

import os
import sys

import numpy as np

for _p in ("/opt/trn_rl_repo", "/opt/pypackages"):
    if _p not in sys.path and os.path.isdir(_p):
        sys.path.append(_p)

# Problem constants (hardcoded per the task contract).
N_FULL = 500000
F = 256
NUM_COMMS = 1000
EPS = 1e-12
M = 8               # cores
P = 128             # partitions
NC_CHUNKS = 8       # community chunks of 128 (8*128 = 1024 >= 1000)
AR_CHUNKS = 2       # community chunks per all-reduce group
XB = 32             # phase-1 x tiles per streaming DMA (32*512B = 16KB lines)
NB = 512            # phase-2 nodes per matmul (psum bank = 512 fp32)
OSTG = 4096         # phase-2 out staging columns per DMA (8KB lines)

# Stash of the most recent run's BassKernelResults (for test harnesses).
LAST_RESULTS = None


def _host_prep(x, community):
    """Build per-core device inputs. Returns (in_maps, plan)."""
    import ml_dtypes

    x = np.ascontiguousarray(np.asarray(x, dtype=np.float32))
    community = np.asarray(community).astype(np.int64)
    n = x.shape[0]
    assert n % M == 0
    nl = n // M

    comm_sh = community.reshape(M, nl)
    perms = np.argsort(comm_sh, axis=1, kind="stable")
    comm_sorted = np.take_along_axis(comm_sh, perms, axis=1)

    # per (core, chunk) node counts
    chunk_ids = comm_sorted >> 7  # // 128
    cnts = np.zeros((M, NC_CHUNKS), dtype=np.int64)
    for m in range(M):
        bc = np.bincount(chunk_ids[m], minlength=NC_CHUNKS)
        cnts[m] = bc[:NC_CHUNKS]
    t_k = np.maximum(1, -(-cnts.max(axis=0) // P))  # ceil, shared by all cores
    t_total = int(t_k.sum())
    chunk_of_tile = np.repeat(np.arange(NC_CHUNKS), t_k)
    tile_off = np.concatenate([[0], np.cumsum(t_k)])  # tile index base per chunk

    # counts -> 1/max(cnt, eps), [p, k] layout (community id = k*128 + p)
    cnt_full = np.bincount(community, minlength=NUM_COMMS).astype(np.float32)
    inv_pad = np.zeros((NC_CHUNKS * P,), np.float32)
    inv_pad[:NUM_COMMS] = 1.0 / np.maximum(cnt_full, np.float32(EPS))
    invc = np.ascontiguousarray(inv_pad.reshape(NC_CHUNKS, P).T)  # [128, 8]

    iota = np.ascontiguousarray(
        np.tile(np.arange(P, dtype=ml_dtypes.bfloat16), (P, 1))
    )  # [128, 128] bf16, each row 0..127

    arange_p = np.arange(P, dtype=np.int32)
    in_maps = []
    origs = []
    for m in range(M):
        x_m = x[m * nl : (m + 1) * nl]
        xs = np.zeros((t_total * P, F), dtype=np.float32)
        locid = np.full((t_total * P,), -1.0, dtype=np.float32)
        orig = np.full((t_total * P,), -1, dtype=np.int64)
        start = 0
        for k in range(NC_CHUNKS):
            c = int(cnts[m, k])
            row = int(tile_off[k]) * P
            sel = perms[m, start : start + c]
            xs[row : row + c] = x_m[sel]
            orig[row : row + c] = sel
            locid[row : row + c] = comm_sorted[m, start : start + c] - k * P
            start += c
        origs.append(orig)

        # xs: [128, T, 256] bf16 -- xs_dev[p, t, f] = x_sorted[t*128+p, f]
        xs_dev = np.ascontiguousarray(
            xs.reshape(t_total, P, F).transpose(1, 0, 2).astype(ml_dtypes.bfloat16)
        )
        # locid: [128, T] fp32 (per-partition scalar for phase-1 is_equal)
        locid_t = np.ascontiguousarray(locid.reshape(t_total, P).T)
        # BT: [128, NT] fp8e4 -- BT[c, n] = 1 iff locid[n] == c (sorted order)
        bt = (locid.astype(np.int32)[None, :] == arange_p[:, None]).astype(
            ml_dtypes.float8_e4m3
        )
        in_maps.append(
            {
                "xs": xs_dev,
                "locid": locid_t,
                "bt": np.ascontiguousarray(bt),
                "iota": iota,
                "invc": invc,
            }
        )

    plan = {
        "nl": nl,
        "t_k": [int(v) for v in t_k],
        "t_total": t_total,
        "chunk_of_tile": [int(v) for v in chunk_of_tile],
        "tile_off": [int(v) for v in tile_off],
        "origs": origs,
    }
    return in_maps, plan


def _build_program(plan, use_collective=True):
    from concourse import bacc, mybir, tile

    t_total = plan["t_total"]
    chunk_of_tile = plan["chunk_of_tile"]
    tile_off = plan["tile_off"]
    nt = t_total * P

    dt = mybir.dt
    nc = bacc.Bacc("TRN2", target_bir_lowering=False, debug=False, num_devices=M)

    xs = nc.dram_tensor("xs", [P, t_total, F], dt.bfloat16, kind="ExternalInput")
    locid = nc.dram_tensor("locid", [P, t_total], dt.float32, kind="ExternalInput")
    btd = nc.dram_tensor("bt", [P, nt], dt.float8e4, kind="ExternalInput")
    iota = nc.dram_tensor("iota", [P, P], dt.bfloat16, kind="ExternalInput")
    invc = nc.dram_tensor("invc", [P, NC_CHUNKS], dt.float32, kind="ExternalInput")
    out = nc.dram_tensor("out", [2 * P, nt], dt.bfloat16, kind="ExternalOutput")

    n_ar = NC_CHUNKS // AR_CHUNKS
    arw = AR_CHUNKS * F  # free width of one all-reduce group

    with tile.TileContext(nc) as tc:
        with (
            tc.tile_pool(name="const", bufs=1) as constp,
            tc.tile_pool(name="xsp", bufs=3) as xsp,
            tc.tile_pool(name="btp", bufs=3) as btp,
            tc.tile_pool(name="bp", bufs=8) as bp,
            tc.tile_pool(name="acc", bufs=1) as accp,
            tc.tile_pool(name="outp", bufs=4) as outp,
            tc.tile_pool(name="ps1", bufs=2, space="PSUM") as ps1,
            tc.tile_pool(name="ps2", bufs=4, space="PSUM") as ps2,
            tc.tile_pool(name="dram", bufs=1, space="DRAM") as dramp,
        ):
            iota_t = constp.tile([P, P], dt.bfloat16)
            nc.sync.dma_start(out=iota_t[:], in_=iota.ap())
            locid_t = constp.tile([P, t_total], dt.float32)
            nc.sync.dma_start(out=locid_t[:], in_=locid.ap())
            invc_t = constp.tile([P, NC_CHUNKS], dt.float32)
            nc.sync.dma_start(out=invc_t[:], in_=invc.ap())

            comm_sum = accp.tile([P, NC_CHUNKS * F], dt.float32)
            mean_f32 = accp.tile([P, NC_CHUNKS * F], dt.float32)
            mean_bf = accp.tile([P, NC_CHUNKS * F], dt.bfloat16)
            ar_in = [
                dramp.tile([P, arw], dt.float32, name=f"ar_in{g}")
                for g in range(n_ar)
            ]
            ar_out = [
                dramp.tile([P, arw], dt.float32, name=f"ar_out{g}")
                for g in range(n_ar)
            ]

            # ---- phase 1: streamed one-hot matmul segment sums ----
            xsb = None
            psum_t = None
            for t in range(t_total):
                if t % XB == 0:
                    w = min(XB, t_total - t)
                    xsb = xsp.tile([P, XB * F], dt.bfloat16, tag="xsb")
                    nc.sync.dma_start(
                        out=xsb[:, : w * F].rearrange("p (b f) -> p b f", b=w),
                        in_=xs.ap()[:, t : t + w, :],
                    )
                k = chunk_of_tile[t]
                first = t == tile_off[k]
                last = t == tile_off[k + 1] - 1
                if first:
                    psum_t = ps1.tile([P, F], dt.float32)
                b_t = bp.tile([P, P], dt.bfloat16, tag="b")
                nc.vector.tensor_scalar(
                    b_t[:],
                    iota_t[:],
                    locid_t[:, t : t + 1],
                    None,
                    mybir.AluOpType.is_equal,
                )
                j = t % XB
                nc.tensor.matmul(
                    psum_t[:],
                    lhsT=b_t[:],
                    rhs=xsb[:, j * F : (j + 1) * F],
                    start=first,
                    stop=last,
                )
                if last:
                    nc.scalar.copy(
                        out=comm_sum[:, k * F : (k + 1) * F], in_=psum_t[:]
                    )
                    if (k + 1) % AR_CHUNKS == 0:
                        # Collective chain lives entirely on SP(ar_in) +
                        # GpSimd so DVE/ACT/PE never wait on the AR.
                        g = k // AR_CHUNKS
                        lo = g * arw
                        nc.sync.dma_start(
                            out=ar_in[g], in_=comm_sum[:, lo : lo + arw]
                        )
                        if use_collective:
                            nc.gpsimd.collective_compute(
                                "AllReduce",
                                mybir.AluOpType.add,
                                replica_groups=[list(range(M))],
                                ins=[ar_in[g].opt()],
                                outs=[ar_out[g].opt()],
                            )
                            nc.gpsimd.dma_start(
                                out=mean_f32[:, lo : lo + arw], in_=ar_out[g]
                            )
                        else:
                            nc.gpsimd.dma_start(
                                out=mean_f32[:, lo : lo + arw], in_=ar_in[g]
                            )
                        for kk in range(g * AR_CHUNKS, (g + 1) * AR_CHUNKS):
                            nc.gpsimd.tensor_scalar(
                                mean_bf[:, kk * F : (kk + 1) * F],
                                mean_f32[:, kk * F : (kk + 1) * F],
                                invc_t[:, kk : kk + 1],
                                None,
                                mybir.AluOpType.mult,
                            )

            # ---- phase 2: broadcast means back to (sorted) nodes ----
            # outT[f, n] = mean_chunk[c, f].T @ BT[c, n]; mean slice is the
            # stationary operand, BT streams in wide fp8 slices.
            ncopy = 0
            for k in range(NC_CHUNKS):
                lo = tile_off[k] * P
                hi = tile_off[k + 1] * P
                btb = btp.tile([P, hi - lo], dt.float8e4, tag="btb")
                nc.sync.dma_start(out=btb[:], in_=btd.ap()[:, lo:hi])
                for h in range(2):
                    stg = None
                    fill = 0
                    base = lo
                    for n0 in range(lo, hi, NB):
                        w = min(NB, hi - n0)
                        if fill == 0:
                            stg = outp.tile([P, OSTG], dt.bfloat16, tag="stg")
                            base = n0
                        pso = ps2.tile([P, NB], dt.float32, tag="pso")
                        nc.tensor.matmul(
                            pso[:, :w],
                            lhsT=mean_bf[:, k * F + h * P : k * F + (h + 1) * P],
                            rhs=btb[:, n0 - lo : n0 - lo + w],
                            start=True,
                            stop=True,
                        )
                        dst = stg[:, fill : fill + w]
                        if ncopy % 2 == 0:
                            nc.scalar.copy(out=dst, in_=pso[:, :w])
                        else:
                            nc.vector.tensor_copy(out=dst, in_=pso[:, :w])
                        ncopy += 1
                        fill += w
                        if fill == OSTG or n0 + w == hi:
                            nc.sync.dma_start(
                                out=out.ap()[h * P : (h + 1) * P, base : base + fill],
                                in_=stg[:, :fill],
                            )
                            fill = 0

    nc.compile()
    return nc


def kernel(x, community):
    global LAST_RESULTS
    from concourse.bass_utils import run_bass_kernel_spmd

    in_maps, plan = _host_prep(x, community)
    nc = _build_program(plan)
    res = run_bass_kernel_spmd(nc, in_maps, core_ids=list(range(M)))
    LAST_RESULTS = res
    nl = plan["nl"]
    outs = []
    for m in range(M):
        od = np.asarray(res.results[m]["out"])  # [256, NT] bf16, outT
        out_sorted = od.T.astype(np.float32)  # [NT, 256]
        orig = plan["origs"][m]
        valid = orig >= 0
        out_m = np.empty((nl, F), dtype=np.float32)
        out_m[orig[valid]] = out_sorted[valid]
        outs.append(out_m)
    return np.concatenate(outs, axis=0)
